# revision 39
# baseline (speedup 1.0000x reference)
"""Sliding-window multi-head attention on 8 Trainium2 NeuronCores.

Sharding: tensor-parallel over heads. 16 heads -> 2 heads per core.
Each core computes q/k/v projections for its 2 heads (d' = 128 dims),
banded (window=256) attention for those heads over all tokens, and a
partial output projection (its 128 columns of Wo). Host sums the 8
partials and adds the bias.

All matmul operands are bf16 (same PE throughput as f32r in the cost
model, half the DMA bytes, faster DVE); PSUM accumulation stays f32.
v is projected directly token-major (lhsT = x chunk, rhs = Wv chunk)
so no PE transposes are needed.

The attention phase is ACT-bound (64 exps ~32us vs ~22us of attention
PE work), so batch 1's projection chunks are split into quarter-chunk
"filler" parts and woven into batch 0's attention j-loop: every j gets
~1.3us of projection matmuls between the two heads' PV groups, keeping
PE ahead of ACT. Batch 0's Wo drains also run there; batch 1's drain
inside its own attention js plus a short 3-engine tail.

DMA instructions are fused (one per 512-token x chunk, one per
8-chunk output group; x/outT use a [128 partition, 8, T] layout so a
single DMA's dims line up) because each DMA costs ~0.6us serialized
HWDGE descriptor-generation plus ~0.6us SP-sequencer config.

Softmax normalization is three-staged across js: reciprocal+copy of
h0 when a u chunk completes, recip+copy h1 and the 1/Z broadcast
matmul one j later, the in-place scale at the top of the j after --
so the PE's in-order queue never waits on DVE latency.

PSUM (8 banks): psA x2 (score tiles + 1/Z broadcast), psU x4 (PV
accumulators), psT x2 (proj q/k/v dsts + wo chunks).
"""

import sys

sys.path.insert(0, "/opt/trn_rl_repo")

from contextlib import ExitStack

import numpy as np
import ml_dtypes

import concourse.bass as bass
import concourse.tile as tile
from concourse import bacc, mybir
from concourse.bass_utils import run_bass_kernel_spmd

F32 = mybir.dt.float32
BF16 = mybir.dt.bfloat16
ACT_EXP = mybir.ActivationFunctionType.Exp

N_CORES = 8
B, S, E = 2, 2048, 1024
H, D = 16, 64
T = B * S                # 4096 tokens total
NB = S // 128            # 16 key/query blocks per batch
PADW = S + 256           # 2304: padded q width per batch
WIN = 384                # q-window per key block (3 blocks)
WOFF = {"wq": 0, "wk": 1024, "wv": 2048, "wo": 3072}


class _Ctx:
    pass


LABELS = {}
WO_DMA_UNITS = set()


def _lab(inst, label):
    for obj in (inst, getattr(inst, "ins", None)):
        name = getattr(obj, "name", None)
        if isinstance(name, str):
            LABELS[name] = label
            break
    return inst


def _emit(tc, io):
    nc = tc.nc
    with ExitStack() as ctx:
        const = ctx.enter_context(tc.tile_pool(name="const", bufs=1))
        big = ctx.enter_context(tc.tile_pool(name="big", bufs=1))
        xpool = ctx.enter_context(tc.tile_pool(name="xload", bufs=4))
        expool = ctx.enter_context(tc.tile_pool(name="expool", bufs=10))
        rzpool = ctx.enter_context(tc.tile_pool(name="rzpool", bufs=6))
        ostage = ctx.enter_context(tc.tile_pool(name="ostage", bufs=4))
        psA = ctx.enter_context(tc.tile_pool(name="psA", bufs=2, space="PSUM"))
        psU = ctx.enter_context(tc.tile_pool(name="psU", bufs=4, space="PSUM"))
        psT = ctx.enter_context(tc.tile_pool(name="psT", bufs=2, space="PSUM"))

        g = _Ctx()

        # ---- constants (wq first so proj can start asap) ----------------
        wpack = const.tile([128, 4096], BF16, tag="wpack")
        nc.sync.dma_start(wpack[:, 0:256], io["wpack"][:, 0:256])
        nc.sync.dma_start(wpack[:, 256:1024], io["wpack"][:, 256:1024])
        mpack = const.tile([128, 3 * WIN], BF16, tag="mpack")
        sel = const.tile([1, 256], BF16, tag="sel")

        g.w = lambda kind, e: wpack[:, WOFF[kind] + 128 * e : WOFF[kind] + 128 * e + 128]
        g.mask = lambda i: mpack[:, WIN * i : WIN * i + WIN]
        g.sel = sel

        # ---- persistent activation buffers -----------------------------
        g.qTp = big.tile([128, B, PADW], BF16, tag="qTp")
        g.kT = big.tile([128, T], BF16, tag="kT")
        # vA: [128 tok-in-block, 32 tok-blocks, 2 heads, 64 dims + 2 ones]
        g.vA = big.tile([128, 32, 2, 66], BF16, tag="vA")
        g.aoT = big.tile([128, T], BF16, tag="aoT")

        def load_kv_weights():
            nc.sync.dma_start(wpack[:, 1024:2048], io["wpack"][:, 1024:2048])
            nc.sync.dma_start(wpack[:, 2048:3072], io["wpack"][:, 2048:3072])

        def load_rest_consts():
            nc.sync.dma_start(wpack[:, 3072:4096], io["wpack"][:, 3072:4096])
            nc.sync.dma_start(mpack[:], io["mpack"][:])
            nc.sync.dma_start(sel[:], io["sel"][:])
            nc.sync.dma_start(g.qTp[:, :, 0:128], io["qzero"][:, :, 0, :])
            nc.sync.dma_start(g.qTp[:, :, S + 128 : PADW], io["qzero"][:, :, 1, :])
            nc.sync.dma_start(g.vA[:, :, :, 64:66], io["vones"][:])

        # ---- x chunk loads ---------------------------------------------
        g.xt = {}

        def emit_x_dma(n, split=False):
            xt = xpool.tile([128, 8, 512], BF16, tag="xt", name="xt")
            if split:
                queues = (nc.scalar, nc.sync, nc.scalar, nc.sync)
                for i, e in enumerate(range(0, 8, 2)):
                    queues[i].dma_start(
                        xt[:, e : e + 2, :], io["xT"][:, e : e + 2, 512 * n : 512 * n + 512]
                    )
            else:
                nc.sync.dma_start(xt[:], io["xT"][:, :, 512 * n : 512 * n + 512])
            g.xt[n] = xt

        # ---- wo drain: one osg staging tile + one fused DMA per n ------
        g.wo_ready = []          # list of n (each is 8 m-units)
        g.wo_state = None        # (n, next_m, osg_tile)
        g.ost_n = 0

        def drain_wo(k, engines=("vector", "scalar"), mode="copy", pools=None):
            for _ in range(k):
                if g.wo_state is None:
                    if not g.wo_ready:
                        return
                    n = g.wo_ready.pop(0)
                    osg = None
                    g.wo_state = (n, 0, osg)
                n, m, osg = g.wo_state
                if pools is None:
                    wps = psT.tile([128, 512], F32, tag="t", name="wps")
                elif (g.ost_n // 1) % 2:
                    wps = psA.tile([128, 512], F32, tag="sa", name="wps")
                else:
                    wps = psT.tile([128, 512], F32, tag="t", name="wps")
                _lab(nc.tensor.matmul(
                    wps[:], g.w("wo", m), g.aoT[:, 512 * n : 512 * n + 512],
                    start=True, stop=True,
                ), f"wo-n{n}m{m}")
                if mode == "dma":
                    # drain straight from PSUM to dram (f32) on an idle
                    # sequencer queue; no ACT/DVE copy work
                    WO_DMA_UNITS.add((n, m))
                    q = nc.scalar if (g.ost_n % 2) else nc.vector
                    g.ost_n += 1
                    q.dma_start(
                        io["outT32"][:, m, 512 * n : 512 * n + 512], wps[:]
                    )
                else:
                    if osg is None:
                        osg = ostage.tile([128, 8, 512], BF16, tag="osg", name="osg")
                        g.wo_state = (n, m, osg)
                    eng = engines[g.ost_n % len(engines)]
                    g.ost_n += 1
                    if eng == "scalar":
                        nc.scalar.copy(osg[:, m, :], wps[:])
                    else:
                        nc.vector.tensor_copy(osg[:, m, :], wps[:])
                if m == 7:
                    if osg is not None:
                        nc.sync.dma_start(
                            io["outT"][:, :, 512 * n : 512 * n + 512], osg[:]
                        )
                    g.wo_state = None
                else:
                    g.wo_state = (n, m + 1, osg)

        # ---- projection, in quarter-chunk parts ------------------------
        def proj_parts(n):
            b, cn = divmod(n, 4)

            def part_qk(kind):
                def run():
                    xt = g.xt[n]
                    ps = psT.tile([128, 512], F32, tag="t", name="ps")
                    for e in range(8):
                        _lab(nc.tensor.matmul(
                            ps[:], g.w(kind, e), xt[:, e, :],
                            start=(e == 0), stop=(e == 7),
                        ), f"proj-{kind}-n{n}e{e}")
                    if kind == "wq":
                        o = 128 + 512 * cn
                        nc.scalar.copy(g.qTp[:, b, o : o + 512], ps[:])
                    else:
                        nc.vector.tensor_copy(g.kT[:, 512 * n : 512 * n + 512], ps[:])
                return run

            # v split in two parts sharing one psum tile; each 128-token
            # sub-block's 8-step accumulation is contiguous (u-outer)
            vps = [None]

            def part_v(half):
                def run():
                    xt = g.xt[n]
                    if half == 0:
                        vps[0] = psT.tile([128, 4, 128], F32, tag="t", name="psv")
                    psv = vps[0]
                    for u in range(2 * half, 2 * half + 2):
                        for e in range(8):
                            _lab(nc.tensor.matmul(
                                psv[:, u, :],
                                xt[:, e, 128 * u : 128 * u + 128],
                                g.w("wv", e),
                                start=(e == 0), stop=(e == 7),
                                skip_group_check=True,
                            ), f"proj-v-n{n}e{e}u{u}")
                    if half == 1:
                        nc.vector.tensor_copy(
                            g.vA[:, 4 * n : 4 * n + 4, :, 0:64], psv[:]
                        )
                return run

            return [part_qk("wq"), part_qk("wk"), part_v(0), part_v(1)]

        def proj_chunk(n):
            for p in proj_parts(n):
                p()

        # ---- attention stream for one batch (generator over j) ---------
        def attn_gen(b):
            def emit_score(j, h):
                sT = psA.tile([128, 512], F32, tag="sa", name="sT")
                _lab(nc.tensor.matmul(
                    sT[:, 0:WIN],
                    g.kT[64 * h : 64 * h + 64, S * b + 128 * j : S * b + 128 * j + 128],
                    g.qTp[64 * h : 64 * h + 64, b, 128 * j : 128 * j + WIN],
                    start=True, stop=True,
                ), f"score-b{b}j{j}h{h}")
                ex = expool.tile([128, WIN], BF16, tag="ex", name="ex")
                nc.scalar.activation(ex[:], sT[:, 0:WIN], ACT_EXP)
                mi = 0 if j == 0 else (2 if j == NB - 1 else 1)
                nc.vector.tensor_mul(ex[:], ex[:], g.mask(mi))
                return ex

            def stage_a(c, u0, u1):
                rz0 = rzpool.tile([1, 512], BF16, tag="rz", name="rz0")
                rz1 = rzpool.tile([1, 512], BF16, tag="rz", name="rz1")
                with nc.allow_low_precision(reason="bf16 softmax scale"):
                    nc.vector.reciprocal(rz0[:], u0[64:65, :])
                    nc.vector.reciprocal(rz1[:], u1[64:65, :])
                dst = g.aoT[:, S * b + 512 * c : S * b + 512 * c + 512]
                nc.scalar.copy(dst[0:64, :], u0[0:64, :])
                return (c, u1, rz0, rz1)

            def stage_b(c, u1, rz0, rz1):
                dst = g.aoT[:, S * b + 512 * c : S * b + 512 * c + 512]
                nc.vector.tensor_copy(dst[64:128, :], u1[0:64, :])
                zr = psA.tile([128, 512], F32, tag="sa", name="zr")
                _lab(nc.tensor.matmul(zr[:], g.sel[:, 0:128], rz0[:],
                                      start=True, stop=False), f"zr-b{b}c{c}a")
                _lab(nc.tensor.matmul(zr[:], g.sel[:, 128:256], rz1[:],
                                      start=False, stop=True), f"zr-b{b}c{c}b")
                return (c, zr)

            def stage_c(c, zr):
                dst = g.aoT[:, S * b + 512 * c : S * b + 512 * c + 512]
                nc.vector.tensor_mul(dst, dst, zr[:])
                g.wo_ready.append(4 * b + c)

            def pv(j, h):
                qlo_w, qhi_w = 128 * (j - 1), 128 * (j + 2)
                tb = NB * b + j
                for c in sorted({max(qlo_w, 0) // 512, (min(qhi_w, S) - 1) // 512}):
                    plo = max(qlo_w, 512 * c, 0)
                    phi = min(qhi_w, 512 * c + 512, S)
                    if plo >= phi:
                        continue
                    if (h, c) not in umap:
                        umap[(h, c)] = psU.tile([65, 512], F32, tag="u", name="u")
                        fresh.add((h, c))
                    _lab(nc.tensor.matmul(
                        umap[(h, c)][:, plo - 512 * c : phi - 512 * c],
                        g.vA[:, tb, h, 0:65],
                        ex_cur[h][:, plo - qlo_w : phi - qlo_w],
                        start=((h, c) in fresh), stop=(j == min(4 * c + 4, NB - 1)),
                        skip_group_check=True,
                    ), f"pv-b{b}j{j}h{h}c{c}")
                    fresh.discard((h, c))

            umap, fresh, pend_b, pend_c = {}, set(), [], []
            ex_cur = [emit_score(0, h) for h in (0, 1)]
            for j in range(NB):
                while pend_c:
                    stage_c(*pend_c.pop(0))
                ex_nxt = [emit_score(j + 1, h) for h in (0, 1)] if j + 1 < NB else None
                pv(j, 0)
                yield ("mid", j)
                pv(j, 1)
                while pend_b:
                    pend_c.append(stage_b(*pend_b.pop(0)))
                for c in range(4):
                    if (0, c) in umap and j == min(4 * c + 4, NB - 1):
                        pend_b.append(stage_a(c, umap.pop((0, c)), umap.pop((1, c))))
                ex_cur = ex_nxt
                yield ("end", j)
            while pend_b:
                pend_c.append(stage_b(*pend_b.pop(0)))
            while pend_c:
                stage_c(*pend_c.pop(0))

        # ---- schedule ---------------------------------------------------
        # proj(0..3) whole; then batch-0 attention with proj(4..7) parts
        # as fillers (one quarter-chunk per j, between the PV groups);
        # then batch-1 attention with 2 wo drains per j; then tail.
        emit_x_dma(0, split=True)
        load_kv_weights()
        emit_x_dma(1)
        proj_chunk(0)
        emit_x_dma(2)
        proj_chunk(1)
        emit_x_dma(3)
        load_rest_consts()
        proj_chunk(2)
        emit_x_dma(4)
        proj_chunk(3)

        parts = []
        for n in range(4, 8):
            parts.extend(proj_parts(n))
        dma_at = {2: 5, 6: 6, 10: 7}
        for ev, j in attn_gen(0):
            if ev == "mid":
                if j in dma_at:
                    emit_x_dma(dma_at[j])
                parts[j]()
            else:
                if j >= 6:
                    drain_wo(2 if j < 10 else 3, engines=("scalar", "vector"))

        for ev, j in attn_gen(1):
            if ev == "end":
                drain_wo(1, engines=("vector", "scalar"))
        for _ in range(40):
            drain_wo(2, engines=("scalar", "vector"), pools="both")
        if "dbgQ" in io:
            nc.sync.dma_start(io["dbgQ"][:], g.qTp[:])
            nc.sync.dma_start(io["dbgK"][:], g.kT[:])
            nc.sync.dma_start(io["dbgV"][:], g.vA[:])
            nc.sync.dma_start(io["dbgA"][:], g.aoT[:])


def build_program():
    WO_DMA_UNITS.clear()
    nc = bacc.Bacc("TRN2", target_bir_lowering=False, debug=False, num_devices=N_CORES)
    io = {}

    def inp(name, shape):
        io[name] = nc.dram_tensor(name, shape, BF16, kind="ExternalInput").ap()

    inp("xT", [128, 8, T])
    inp("wpack", [128, 4096])
    inp("mpack", [128, 3 * WIN])
    inp("sel", [1, 256])
    inp("qzero", [128, B, 2, 128])
    inp("vones", [128, 32, 2, 2])
    io["outT"] = nc.dram_tensor("outT", [128, 8, T], BF16, kind="ExternalOutput").ap()
    import os
    if os.environ.get("KDBG"):
        io["dbgQ"] = nc.dram_tensor("dbgQ", [128, B, PADW], BF16, kind="ExternalOutput").ap()
        io["dbgK"] = nc.dram_tensor("dbgK", [128, T], BF16, kind="ExternalOutput").ap()
        io["dbgV"] = nc.dram_tensor("dbgV", [128, 32, 2, 66], BF16, kind="ExternalOutput").ap()
        io["dbgA"] = nc.dram_tensor("dbgA", [128, T], BF16, kind="ExternalOutput").ap()
    io["outT32"] = nc.dram_tensor("outT32", [128, 8, T], F32, kind="ExternalOutput").ap()

    with tile.TileContext(nc) as tc:
        _emit(tc, io)
    nc.compile()
    return nc


def _to_bf16(a):
    u = np.ascontiguousarray(a, dtype=np.float32).view(np.uint32)
    r = ((u + 0x7FFF + ((u >> 16) & 1)) >> 16).astype(np.uint16)
    return r.view(ml_dtypes.bfloat16)


def _bf16_to_f32(a):
    u = np.asarray(a).view(np.uint16).astype(np.uint32) << 16
    return u.view(np.float32)


def _host_inputs(x, Wq, Wk, Wv, Wo):
    """Per-core input maps (host-side sharding / relayout)."""
    xf = np.ascontiguousarray(x.reshape(T, E).T)            # [1024, 4096]
    # [p, e, t]: E index = 128e + p
    xT = np.ascontiguousarray(_to_bf16(xf).reshape(8, 128, T).transpose(1, 0, 2))

    band = np.zeros((128, WIN), dtype=np.float32)
    for r in range(128):
        band[r, r : r + 257] = 1.0                           # |q - k| <= 128
    m_left = band.copy()
    m_left[:, :128] = 0.0
    m_right = band.copy()
    m_right[:, 256:] = 0.0
    mpack = _to_bf16(np.concatenate([m_left, band, m_right], axis=1))

    sel = np.zeros((1, 256), dtype=np.float32)
    sel[0, 0:64] = 1.0
    sel[0, 192:256] = 1.0
    sel = _to_bf16(sel)
    qzero = np.zeros((128, B, 2, 128), dtype=ml_dtypes.bfloat16)
    vones = np.ones((128, 32, 2, 2), dtype=ml_dtypes.bfloat16)

    scale = 1.0 / np.sqrt(D)
    in_maps = []
    for c in range(N_CORES):
        rows = slice(128 * c, 128 * c + 128)
        wq = np.ascontiguousarray((Wq[rows, :] * scale).T)   # [1024 e, 128 d']
        wk = np.ascontiguousarray(Wk[rows, :].T)
        wv = np.ascontiguousarray(Wv[rows, :].T)
        wqc = wq.reshape(8, 128, 128)
        wkc = wk.reshape(8, 128, 128)
        wvc = wv.reshape(8, 128, 128)
        woc = Wo[:, rows].T.reshape(128, 8, 128).transpose(1, 0, 2)  # [8,128 d',128 e]
        wpack = np.zeros((128, 4096), dtype=np.float32)
        for e in range(8):
            wpack[:, 0 + 128 * e : 128 * e + 128] = wqc[e]
            wpack[:, 1024 + 128 * e : 1152 + 128 * e] = wkc[e]
            wpack[:, 2048 + 128 * e : 2176 + 128 * e] = wvc[e]
            wpack[:, 3072 + 128 * e : 3200 + 128 * e] = woc[e]
        in_maps.append(
            {
                "xT": xT,
                "wpack": _to_bf16(wpack),
                "mpack": mpack,
                "sel": sel,
                "qzero": qzero,
                "vones": vones,
            }
        )
    return in_maps


_NC_CACHE = None


def kernel(x, Wq, Wk, Wv, Wo, bo):
    global _NC_CACHE
    x = np.asarray(x, dtype=np.float32)
    Wq = np.asarray(Wq, dtype=np.float32)
    Wk = np.asarray(Wk, dtype=np.float32)
    Wv = np.asarray(Wv, dtype=np.float32)
    Wo = np.asarray(Wo, dtype=np.float32)
    bo = np.asarray(bo, dtype=np.float32)

    if _NC_CACHE is None:
        _NC_CACHE = build_program()
    nc = _NC_CACHE

    in_maps = _host_inputs(x, Wq, Wk, Wv, Wo)
    res = run_bass_kernel_spmd(nc, in_maps, core_ids=list(range(N_CORES)))

    acc = np.zeros((E, T), dtype=np.float32)
    for c in range(N_CORES):
        # outT[p, m, t] -> E row = 128m + p
        o = _bf16_to_f32(res.results[c]["outT"]).reshape(128, 8, T)
        acc += o.transpose(1, 0, 2).reshape(E, T)
    out = acc.T + bo[None, :]
    return np.ascontiguousarray(out.reshape(B, S, E))


# revision 42
# speedup vs baseline: 1.0055x; 1.0055x over previous
"""Sliding-window multi-head attention on 8 Trainium2 NeuronCores.

Sharding: tensor-parallel over heads. 16 heads -> 2 heads per core.
Each core computes q/k/v projections for its 2 heads (d' = 128 dims),
banded (window=256) attention for those heads over all tokens, and a
partial output projection (its 128 columns of Wo). Host sums the 8
partials and adds the bias.

All matmul operands are bf16 (same PE throughput as f32r in the cost
model, half the DMA bytes, faster DVE); PSUM accumulation stays f32.
v is projected directly token-major (lhsT = x chunk, rhs = Wv chunk)
so no PE transposes are needed.

The attention phase is ACT-bound (64 exps ~32us vs ~22us of attention
PE work), so batch 1's projection chunks are split into quarter-chunk
"filler" parts and woven into batch 0's attention j-loop: every j gets
~1.3us of projection matmuls between the two heads' PV groups, keeping
PE ahead of ACT. Batch 0's Wo drains also run there; batch 1's drain
inside its own attention js plus a short 3-engine tail.

DMA instructions are fused (one per 512-token x chunk, one per
8-chunk output group; x/outT use a [128 partition, 8, T] layout so a
single DMA's dims line up) because each DMA costs ~0.6us serialized
HWDGE descriptor-generation plus ~0.6us SP-sequencer config.

Softmax normalization is three-staged across js: reciprocal+copy of
h0 when a u chunk completes, recip+copy h1 and the 1/Z broadcast
matmul one j later, the in-place scale at the top of the j after --
so the PE's in-order queue never waits on DVE latency.

PSUM (8 banks): psA x2 (score tiles + 1/Z broadcast), psU x4 (PV
accumulators), psT x2 (proj q/k/v dsts + wo chunks).
"""

import sys

sys.path.insert(0, "/opt/trn_rl_repo")

from contextlib import ExitStack

import numpy as np
import ml_dtypes

import concourse.bass as bass
import concourse.tile as tile
from concourse import bacc, mybir
from concourse.bass_utils import run_bass_kernel_spmd

F32 = mybir.dt.float32
BF16 = mybir.dt.bfloat16
ACT_EXP = mybir.ActivationFunctionType.Exp

N_CORES = 8
B, S, E = 2, 2048, 1024
H, D = 16, 64
T = B * S                # 4096 tokens total
NB = S // 128            # 16 key/query blocks per batch
PADW = S + 256           # 2304: padded q width per batch
WIN = 384                # q-window per key block (3 blocks)
WOFF = {"wq": 0, "wk": 1024, "wv": 2048, "wo": 3072}


class _Ctx:
    pass


LABELS = {}
WO_DMA_UNITS = set()


def _lab(inst, label):
    for obj in (inst, getattr(inst, "ins", None)):
        name = getattr(obj, "name", None)
        if isinstance(name, str):
            LABELS[name] = label
            break
    return inst


def _emit(tc, io):
    nc = tc.nc
    with ExitStack() as ctx:
        const = ctx.enter_context(tc.tile_pool(name="const", bufs=1))
        big = ctx.enter_context(tc.tile_pool(name="big", bufs=1))
        xpool = ctx.enter_context(tc.tile_pool(name="xload", bufs=4))
        expool = ctx.enter_context(tc.tile_pool(name="expool", bufs=10))
        rzpool = ctx.enter_context(tc.tile_pool(name="rzpool", bufs=6))
        ostage = ctx.enter_context(tc.tile_pool(name="ostage", bufs=4))
        psA = ctx.enter_context(tc.tile_pool(name="psA", bufs=2, space="PSUM"))
        psU = ctx.enter_context(tc.tile_pool(name="psU", bufs=4, space="PSUM"))
        psT = ctx.enter_context(tc.tile_pool(name="psT", bufs=2, space="PSUM"))

        g = _Ctx()

        # ---- constants (wq first so proj can start asap) ----------------
        wpack = const.tile([128, 4096], BF16, tag="wpack")
        nc.sync.dma_start(wpack[:, 0:256], io["wpack"][:, 0:256])
        nc.sync.dma_start(wpack[:, 256:1024], io["wpack"][:, 256:1024])
        mpack = const.tile([128, 3 * WIN], BF16, tag="mpack")
        sel = const.tile([1, 256], BF16, tag="sel")

        g.w = lambda kind, e: wpack[:, WOFF[kind] + 128 * e : WOFF[kind] + 128 * e + 128]
        g.mask = lambda i: mpack[:, WIN * i : WIN * i + WIN]
        g.sel = sel

        # ---- persistent activation buffers -----------------------------
        g.qTp = big.tile([128, B, PADW], BF16, tag="qTp")
        g.kT = big.tile([128, T], BF16, tag="kT")
        # vA: [128 tok-in-block, 32 tok-blocks, 2 heads, 64 dims + 2 ones]
        g.vA = big.tile([128, 32, 2, 66], BF16, tag="vA")
        g.aoT = big.tile([128, T], BF16, tag="aoT")

        def load_kv_weights():
            nc.sync.dma_start(wpack[:, 1024:2048], io["wpack"][:, 1024:2048])
            nc.sync.dma_start(wpack[:, 2048:3072], io["wpack"][:, 2048:3072])

        def load_rest_consts():
            nc.sync.dma_start(wpack[:, 3072:4096], io["wpack"][:, 3072:4096])
            nc.sync.dma_start(mpack[:], io["mpack"][:])
            nc.sync.dma_start(sel[:], io["sel"][:])
            nc.sync.dma_start(g.qTp[:, :, 0:128], io["qzero"][:, :, 0, :])
            nc.sync.dma_start(g.qTp[:, :, S + 128 : PADW], io["qzero"][:, :, 1, :])
            nc.sync.dma_start(g.vA[:, :, :, 64:66], io["vones"][:])

        # ---- x chunk loads ---------------------------------------------
        g.xt = {}

        def emit_x_dma(n, split=False):
            xt = xpool.tile([128, 8, 512], BF16, tag="xt", name="xt")
            if split:
                queues = (nc.scalar, nc.scalar, nc.sync, nc.sync)
                for i, e in enumerate(range(0, 8, 2)):
                    queues[i].dma_start(
                        xt[:, e : e + 2, :], io["xT"][:, e : e + 2, 512 * n : 512 * n + 512]
                    )
            else:
                nc.sync.dma_start(xt[:], io["xT"][:, :, 512 * n : 512 * n + 512])
            g.xt[n] = xt

        # ---- wo drain: one osg staging tile + one fused DMA per n ------
        g.wo_ready = []          # list of n (each is 8 m-units)
        g.wo_state = None        # (n, next_m, osg_tile)
        g.ost_n = 0

        def drain_wo(k, engines=("vector", "scalar"), mode="copy", pools=None):
            for _ in range(k):
                if g.wo_state is None:
                    if not g.wo_ready:
                        return
                    n = g.wo_ready.pop(0)
                    osg = None
                    g.wo_state = (n, 0, osg)
                n, m, osg = g.wo_state
                if pools is None:
                    wps = psT.tile([128, 512], F32, tag="t", name="wps")
                elif (g.ost_n // 1) % 2:
                    wps = psA.tile([128, 512], F32, tag="sa", name="wps")
                else:
                    wps = psT.tile([128, 512], F32, tag="t", name="wps")
                _lab(nc.tensor.matmul(
                    wps[:], g.w("wo", m), g.aoT[:, 512 * n : 512 * n + 512],
                    start=True, stop=True,
                ), f"wo-n{n}m{m}")
                if mode == "dma":
                    # drain straight from PSUM to dram (f32) on an idle
                    # sequencer queue; no ACT/DVE copy work
                    WO_DMA_UNITS.add((n, m))
                    q = nc.scalar if (g.ost_n % 2) else nc.vector
                    g.ost_n += 1
                    q.dma_start(
                        io["outT32"][:, m, 512 * n : 512 * n + 512], wps[:]
                    )
                else:
                    if osg is None:
                        osg = ostage.tile([128, 8, 512], BF16, tag="osg", name="osg")
                        g.wo_state = (n, m, osg)
                    eng = engines[g.ost_n % len(engines)]
                    g.ost_n += 1
                    if eng == "scalar":
                        nc.scalar.copy(osg[:, m, :], wps[:])
                    else:
                        nc.vector.tensor_copy(osg[:, m, :], wps[:])
                if m == 7:
                    if osg is not None:
                        nc.sync.dma_start(
                            io["outT"][:, :, 512 * n : 512 * n + 512], osg[:]
                        )
                    g.wo_state = None
                else:
                    g.wo_state = (n, m + 1, osg)

        # ---- projection, in quarter-chunk parts ------------------------
        def proj_parts(n):
            b, cn = divmod(n, 4)

            def part_qk(kind):
                def run():
                    xt = g.xt[n]
                    ps = psT.tile([128, 512], F32, tag="t", name="ps")
                    for e in range(8):
                        _lab(nc.tensor.matmul(
                            ps[:], g.w(kind, e), xt[:, e, :],
                            start=(e == 0), stop=(e == 7),
                        ), f"proj-{kind}-n{n}e{e}")
                    if kind == "wq":
                        o = 128 + 512 * cn
                        nc.scalar.copy(g.qTp[:, b, o : o + 512], ps[:])
                    else:
                        nc.vector.tensor_copy(g.kT[:, 512 * n : 512 * n + 512], ps[:])
                return run

            # v split in two parts sharing one psum tile; each 128-token
            # sub-block's 8-step accumulation is contiguous (u-outer)
            vps = [None]

            def part_v(half):
                def run():
                    xt = g.xt[n]
                    if half == 0:
                        vps[0] = psT.tile([128, 4, 128], F32, tag="t", name="psv")
                    psv = vps[0]
                    for u in range(2 * half, 2 * half + 2):
                        for e in range(8):
                            _lab(nc.tensor.matmul(
                                psv[:, u, :],
                                xt[:, e, 128 * u : 128 * u + 128],
                                g.w("wv", e),
                                start=(e == 0), stop=(e == 7),
                                skip_group_check=True,
                            ), f"proj-v-n{n}e{e}u{u}")
                    if half == 1:
                        nc.vector.tensor_copy(
                            g.vA[:, 4 * n : 4 * n + 4, :, 0:64], psv[:]
                        )
                return run

            return [part_qk("wq"), part_qk("wk"), part_v(0), part_v(1)]

        def proj_chunk(n):
            for p in proj_parts(n):
                p()

        # ---- attention stream for one batch (generator over j) ---------
        def attn_gen(b):
            def emit_score(j, h):
                # edge j-blocks skip their 128 padded columns entirely
                if j == 0:
                    qo, w, mk = 128, 256, mpack[:, 128:384]
                elif j == NB - 1:
                    qo, w, mk = 0, 256, mpack[:, 2 * WIN : 2 * WIN + 256]
                else:
                    qo, w, mk = 0, WIN, g.mask(1)
                sT = psA.tile([128, 512], F32, tag="sa", name="sT")
                _lab(nc.tensor.matmul(
                    sT[:, 0:w],
                    g.kT[64 * h : 64 * h + 64, S * b + 128 * j : S * b + 128 * j + 128],
                    g.qTp[64 * h : 64 * h + 64, b, 128 * j + qo : 128 * j + qo + w],
                    start=True, stop=True,
                ), f"score-b{b}j{j}h{h}")
                ex = expool.tile([128, WIN], BF16, tag="ex", name="ex")
                nc.scalar.activation(ex[:, 0:w], sT[:, 0:w], ACT_EXP)
                nc.vector.tensor_mul(ex[:, 0:w], ex[:, 0:w], mk)
                return ex

            def stage_a(c, u0, u1):
                rz0 = rzpool.tile([1, 512], BF16, tag="rz", name="rz0")
                rz1 = rzpool.tile([1, 512], BF16, tag="rz", name="rz1")
                with nc.allow_low_precision(reason="bf16 softmax scale"):
                    nc.vector.reciprocal(rz0[:], u0[64:65, :])
                    nc.vector.reciprocal(rz1[:], u1[64:65, :])
                dst = g.aoT[:, S * b + 512 * c : S * b + 512 * c + 512]
                nc.scalar.copy(dst[0:64, :], u0[0:64, :])
                return (c, u1, rz0, rz1)

            def stage_b(c, u1, rz0, rz1):
                dst = g.aoT[:, S * b + 512 * c : S * b + 512 * c + 512]
                nc.vector.tensor_copy(dst[64:128, :], u1[0:64, :])
                zr = psA.tile([128, 512], F32, tag="sa", name="zr")
                _lab(nc.tensor.matmul(zr[:], g.sel[:, 0:128], rz0[:],
                                      start=True, stop=False), f"zr-b{b}c{c}a")
                _lab(nc.tensor.matmul(zr[:], g.sel[:, 128:256], rz1[:],
                                      start=False, stop=True), f"zr-b{b}c{c}b")
                return (c, zr)

            def stage_c(c, zr):
                dst = g.aoT[:, S * b + 512 * c : S * b + 512 * c + 512]
                nc.vector.tensor_mul(dst, dst, zr[:])
                g.wo_ready.append(4 * b + c)

            def pv(j, h):
                qlo_w, qhi_w = 128 * (j - 1), 128 * (j + 2)
                ex_off = 0 if j == 0 else qlo_w
                tb = NB * b + j
                for c in sorted({max(qlo_w, 0) // 512, (min(qhi_w, S) - 1) // 512}):
                    plo = max(qlo_w, 512 * c, 0)
                    phi = min(qhi_w, 512 * c + 512, S)
                    if plo >= phi:
                        continue
                    if (h, c) not in umap:
                        umap[(h, c)] = psU.tile([65, 512], F32, tag="u", name="u")
                        fresh.add((h, c))
                    _lab(nc.tensor.matmul(
                        umap[(h, c)][:, plo - 512 * c : phi - 512 * c],
                        g.vA[:, tb, h, 0:65],
                        ex_cur[h][:, plo - ex_off : phi - ex_off],
                        start=((h, c) in fresh), stop=(j == min(4 * c + 4, NB - 1)),
                        skip_group_check=True,
                    ), f"pv-b{b}j{j}h{h}c{c}")
                    fresh.discard((h, c))

            umap, fresh, pend_b, pend_c = {}, set(), [], []
            ex_cur = [emit_score(0, h) for h in (0, 1)]
            for j in range(NB):
                while pend_c:
                    stage_c(*pend_c.pop(0))
                ex_nxt = [emit_score(j + 1, h) for h in (0, 1)] if j + 1 < NB else None
                pv(j, 0)
                yield ("mid", j)
                pv(j, 1)
                while pend_b:
                    pend_c.append(stage_b(*pend_b.pop(0)))
                for c in range(4):
                    if (0, c) in umap and j == min(4 * c + 4, NB - 1):
                        pend_b.append(stage_a(c, umap.pop((0, c)), umap.pop((1, c))))
                ex_cur = ex_nxt
                yield ("end", j)
            while pend_b:
                pend_c.append(stage_b(*pend_b.pop(0)))
            while pend_c:
                stage_c(*pend_c.pop(0))

        # ---- schedule ---------------------------------------------------
        # proj(0..3) whole; then batch-0 attention with proj(4..7) parts
        # as fillers (one quarter-chunk per j, between the PV groups);
        # then batch-1 attention with 2 wo drains per j; then tail.
        emit_x_dma(0, split=True)
        load_kv_weights()
        emit_x_dma(1)
        proj_chunk(0)
        emit_x_dma(2)
        proj_chunk(1)
        emit_x_dma(3)
        load_rest_consts()
        proj_chunk(2)
        emit_x_dma(4)
        proj_chunk(3)

        parts = []
        for n in range(4, 8):
            parts.extend(proj_parts(n))
        dma_at = {2: 5, 6: 6, 10: 7}
        for ev, j in attn_gen(0):
            if ev == "mid":
                if j in dma_at:
                    emit_x_dma(dma_at[j])
                parts[j]()
            else:
                if j >= 6:
                    drain_wo(2 if j < 10 else 3, engines=("scalar", "vector"))

        for ev, j in attn_gen(1):
            if ev == "mid":
                drain_wo(1, engines=("vector", "scalar"))
        for _ in range(40):
            drain_wo(2, engines=("scalar", "vector"), pools="both")
        if "dbgQ" in io:
            nc.sync.dma_start(io["dbgQ"][:], g.qTp[:])
            nc.sync.dma_start(io["dbgK"][:], g.kT[:])
            nc.sync.dma_start(io["dbgV"][:], g.vA[:])
            nc.sync.dma_start(io["dbgA"][:], g.aoT[:])


def build_program():
    WO_DMA_UNITS.clear()
    nc = bacc.Bacc("TRN2", target_bir_lowering=False, debug=False, num_devices=N_CORES)
    io = {}

    def inp(name, shape):
        io[name] = nc.dram_tensor(name, shape, BF16, kind="ExternalInput").ap()

    inp("xT", [128, 8, T])
    inp("wpack", [128, 4096])
    inp("mpack", [128, 3 * WIN])
    inp("sel", [1, 256])
    inp("qzero", [128, B, 2, 128])
    inp("vones", [128, 32, 2, 2])
    io["outT"] = nc.dram_tensor("outT", [128, 8, T], BF16, kind="ExternalOutput").ap()
    import os
    if os.environ.get("KDBG"):
        io["dbgQ"] = nc.dram_tensor("dbgQ", [128, B, PADW], BF16, kind="ExternalOutput").ap()
        io["dbgK"] = nc.dram_tensor("dbgK", [128, T], BF16, kind="ExternalOutput").ap()
        io["dbgV"] = nc.dram_tensor("dbgV", [128, 32, 2, 66], BF16, kind="ExternalOutput").ap()
        io["dbgA"] = nc.dram_tensor("dbgA", [128, T], BF16, kind="ExternalOutput").ap()
    io["outT32"] = nc.dram_tensor("outT32", [128, 8, T], F32, kind="ExternalOutput").ap()

    with tile.TileContext(nc) as tc:
        _emit(tc, io)
    nc.compile()
    return nc


def _to_bf16(a):
    u = np.ascontiguousarray(a, dtype=np.float32).view(np.uint32)
    r = ((u + 0x7FFF + ((u >> 16) & 1)) >> 16).astype(np.uint16)
    return r.view(ml_dtypes.bfloat16)


def _bf16_to_f32(a):
    u = np.asarray(a).view(np.uint16).astype(np.uint32) << 16
    return u.view(np.float32)


def _host_inputs(x, Wq, Wk, Wv, Wo):
    """Per-core input maps (host-side sharding / relayout)."""
    xf = np.ascontiguousarray(x.reshape(T, E).T)            # [1024, 4096]
    # [p, e, t]: E index = 128e + p
    xT = np.ascontiguousarray(_to_bf16(xf).reshape(8, 128, T).transpose(1, 0, 2))

    band = np.zeros((128, WIN), dtype=np.float32)
    for r in range(128):
        band[r, r : r + 257] = 1.0                           # |q - k| <= 128
    m_left = band.copy()
    m_left[:, :128] = 0.0
    m_right = band.copy()
    m_right[:, 256:] = 0.0
    mpack = _to_bf16(np.concatenate([m_left, band, m_right], axis=1))

    sel = np.zeros((1, 256), dtype=np.float32)
    sel[0, 0:64] = 1.0
    sel[0, 192:256] = 1.0
    sel = _to_bf16(sel)
    qzero = np.zeros((128, B, 2, 128), dtype=ml_dtypes.bfloat16)
    vones = np.ones((128, 32, 2, 2), dtype=ml_dtypes.bfloat16)

    scale = 1.0 / np.sqrt(D)
    in_maps = []
    for c in range(N_CORES):
        rows = slice(128 * c, 128 * c + 128)
        wq = np.ascontiguousarray((Wq[rows, :] * scale).T)   # [1024 e, 128 d']
        wk = np.ascontiguousarray(Wk[rows, :].T)
        wv = np.ascontiguousarray(Wv[rows, :].T)
        wqc = wq.reshape(8, 128, 128)
        wkc = wk.reshape(8, 128, 128)
        wvc = wv.reshape(8, 128, 128)
        woc = Wo[:, rows].T.reshape(128, 8, 128).transpose(1, 0, 2)  # [8,128 d',128 e]
        wpack = np.zeros((128, 4096), dtype=np.float32)
        for e in range(8):
            wpack[:, 0 + 128 * e : 128 * e + 128] = wqc[e]
            wpack[:, 1024 + 128 * e : 1152 + 128 * e] = wkc[e]
            wpack[:, 2048 + 128 * e : 2176 + 128 * e] = wvc[e]
            wpack[:, 3072 + 128 * e : 3200 + 128 * e] = woc[e]
        in_maps.append(
            {
                "xT": xT,
                "wpack": _to_bf16(wpack),
                "mpack": mpack,
                "sel": sel,
                "qzero": qzero,
                "vones": vones,
            }
        )
    return in_maps


_NC_CACHE = None


def kernel(x, Wq, Wk, Wv, Wo, bo):
    global _NC_CACHE
    x = np.asarray(x, dtype=np.float32)
    Wq = np.asarray(Wq, dtype=np.float32)
    Wk = np.asarray(Wk, dtype=np.float32)
    Wv = np.asarray(Wv, dtype=np.float32)
    Wo = np.asarray(Wo, dtype=np.float32)
    bo = np.asarray(bo, dtype=np.float32)

    if _NC_CACHE is None:
        _NC_CACHE = build_program()
    nc = _NC_CACHE

    in_maps = _host_inputs(x, Wq, Wk, Wv, Wo)
    res = run_bass_kernel_spmd(nc, in_maps, core_ids=list(range(N_CORES)))

    acc = np.zeros((E, T), dtype=np.float32)
    for c in range(N_CORES):
        # outT[p, m, t] -> E row = 128m + p
        o = _bf16_to_f32(res.results[c]["outT"]).reshape(128, 8, T)
        acc += o.transpose(1, 0, 2).reshape(E, T)
    out = acc.T + bo[None, :]
    return np.ascontiguousarray(out.reshape(B, S, E))


# revision 43
# speedup vs baseline: 1.0069x; 1.0014x over previous
"""Sliding-window multi-head attention on 8 Trainium2 NeuronCores.

Sharding: tensor-parallel over heads. 16 heads -> 2 heads per core.
Each core computes q/k/v projections for its 2 heads (d' = 128 dims),
banded (window=256) attention for those heads over all tokens, and a
partial output projection (its 128 columns of Wo). Host sums the 8
partials and adds the bias.

All matmul operands are bf16 (same PE throughput as f32r in the cost
model, half the DMA bytes, faster DVE); PSUM accumulation stays f32.
v is projected directly token-major (lhsT = x chunk, rhs = Wv chunk)
so no PE transposes are needed.

The attention phase is ACT-bound (64 exps ~32us vs ~22us of attention
PE work), so batch 1's projection chunks are split into quarter-chunk
"filler" parts and woven into batch 0's attention j-loop: every j gets
~1.3us of projection matmuls between the two heads' PV groups, keeping
PE ahead of ACT. Batch 0's Wo drains also run there; batch 1's drain
inside its own attention js plus a short 3-engine tail.

DMA instructions are fused (one per 512-token x chunk, one per
8-chunk output group; x/outT use a [128 partition, 8, T] layout so a
single DMA's dims line up) because each DMA costs ~0.6us serialized
HWDGE descriptor-generation plus ~0.6us SP-sequencer config.

Softmax normalization is three-staged across js: reciprocal+copy of
h0 when a u chunk completes, recip+copy h1 and the 1/Z broadcast
matmul one j later, the in-place scale at the top of the j after --
so the PE's in-order queue never waits on DVE latency.

PSUM (8 banks): psA x2 (score tiles + 1/Z broadcast), psU x4 (PV
accumulators), psT x2 (proj q/k/v dsts + wo chunks).
"""

import sys

sys.path.insert(0, "/opt/trn_rl_repo")

from contextlib import ExitStack

import numpy as np
import ml_dtypes

import concourse.bass as bass
import concourse.tile as tile
from concourse import bacc, mybir
from concourse.bass_utils import run_bass_kernel_spmd

F32 = mybir.dt.float32
BF16 = mybir.dt.bfloat16
ACT_EXP = mybir.ActivationFunctionType.Exp

N_CORES = 8
B, S, E = 2, 2048, 1024
H, D = 16, 64
T = B * S                # 4096 tokens total
NB = S // 128            # 16 key/query blocks per batch
PADW = S + 256           # 2304: padded q width per batch
WIN = 384                # q-window per key block (3 blocks)
WOFF = {"wq": 0, "wk": 1024, "wv": 2048, "wo": 3072}


class _Ctx:
    pass


LABELS = {}
WO_DMA_UNITS = set()


def _lab(inst, label):
    for obj in (inst, getattr(inst, "ins", None)):
        name = getattr(obj, "name", None)
        if isinstance(name, str):
            LABELS[name] = label
            break
    return inst


def _emit(tc, io):
    nc = tc.nc
    with ExitStack() as ctx:
        const = ctx.enter_context(tc.tile_pool(name="const", bufs=1))
        big = ctx.enter_context(tc.tile_pool(name="big", bufs=1))
        xpool = ctx.enter_context(tc.tile_pool(name="xload", bufs=4))
        expool = ctx.enter_context(tc.tile_pool(name="expool", bufs=10))
        rzpool = ctx.enter_context(tc.tile_pool(name="rzpool", bufs=6))
        ostage = ctx.enter_context(tc.tile_pool(name="ostage", bufs=4))
        psA = ctx.enter_context(tc.tile_pool(name="psA", bufs=2, space="PSUM"))
        psU = ctx.enter_context(tc.tile_pool(name="psU", bufs=4, space="PSUM"))
        psT = ctx.enter_context(tc.tile_pool(name="psT", bufs=2, space="PSUM"))

        g = _Ctx()

        # ---- constants (wq first so proj can start asap) ----------------
        wpack = const.tile([128, 4096], BF16, tag="wpack")
        nc.sync.dma_start(wpack[:, 0:256], io["wpack"][:, 0:256])
        nc.sync.dma_start(wpack[:, 256:1024], io["wpack"][:, 256:1024])
        mpack = const.tile([128, 3 * WIN], BF16, tag="mpack")
        sel = const.tile([1, 256], BF16, tag="sel")

        g.w = lambda kind, e: wpack[:, WOFF[kind] + 128 * e : WOFF[kind] + 128 * e + 128]
        g.mask = lambda i: mpack[:, WIN * i : WIN * i + WIN]
        g.sel = sel

        # ---- persistent activation buffers -----------------------------
        g.qTp = big.tile([128, B, PADW], BF16, tag="qTp")
        g.kT = big.tile([128, T], BF16, tag="kT")
        # vA: [128 tok-in-block, 32 tok-blocks, 2 heads, 64 dims + 2 ones]
        g.vA = big.tile([128, 32, 2, 66], BF16, tag="vA")
        g.aoT = big.tile([128, T], BF16, tag="aoT")

        def load_kv_weights():
            nc.sync.dma_start(wpack[:, 1024:2048], io["wpack"][:, 1024:2048])
            nc.sync.dma_start(wpack[:, 2048:3072], io["wpack"][:, 2048:3072])

        def load_rest_consts():
            nc.sync.dma_start(wpack[:, 3072:4096], io["wpack"][:, 3072:4096])
            nc.sync.dma_start(mpack[:], io["mpack"][:])
            nc.sync.dma_start(sel[:], io["sel"][:])
            nc.sync.dma_start(g.qTp[:, :, 0:128], io["qzero"][:, :, 0, :])
            nc.sync.dma_start(g.qTp[:, :, S + 128 : PADW], io["qzero"][:, :, 1, :])
            nc.sync.dma_start(g.vA[:, :, :, 64:66], io["vones"][:])

        # ---- x chunk loads ---------------------------------------------
        g.xt = {}

        def emit_x_dma(n, split=False):
            xt = xpool.tile([128, 8, 512], BF16, tag="xt", name="xt")
            if split:
                queues = (nc.scalar, nc.scalar, nc.sync, nc.sync)
                for i, e in enumerate(range(0, 8, 2)):
                    queues[i].dma_start(
                        xt[:, e : e + 2, :], io["xT"][:, e : e + 2, 512 * n : 512 * n + 512]
                    )
            else:
                nc.sync.dma_start(xt[:], io["xT"][:, :, 512 * n : 512 * n + 512])
            g.xt[n] = xt

        # ---- wo drain: one osg staging tile + one fused DMA per n ------
        g.wo_ready = []          # list of n (each is 8 m-units)
        g.wo_state = None        # (n, next_m, osg_tile)
        g.ost_n = 0

        def drain_wo(k, engines=("vector", "scalar"), mode="copy", pools=None):
            for _ in range(k):
                if g.wo_state is None:
                    if not g.wo_ready:
                        return
                    n = g.wo_ready.pop(0)
                    osg = None
                    g.wo_state = (n, 0, osg)
                n, m, osg = g.wo_state
                if pools is None:
                    wps = psT.tile([128, 512], F32, tag="t", name="wps")
                elif (g.ost_n // 1) % 2:
                    wps = psA.tile([128, 512], F32, tag="sa", name="wps")
                else:
                    wps = psT.tile([128, 512], F32, tag="t", name="wps")
                _lab(nc.tensor.matmul(
                    wps[:], g.w("wo", m), g.aoT[:, 512 * n : 512 * n + 512],
                    start=True, stop=True,
                ), f"wo-n{n}m{m}")
                if mode == "dma":
                    # drain straight from PSUM to dram (f32) on an idle
                    # sequencer queue; no ACT/DVE copy work
                    WO_DMA_UNITS.add((n, m))
                    q = nc.scalar if (g.ost_n % 2) else nc.vector
                    g.ost_n += 1
                    q.dma_start(
                        io["outT32"][:, m, 512 * n : 512 * n + 512], wps[:]
                    )
                else:
                    if osg is None:
                        osg = ostage.tile([128, 8, 512], BF16, tag="osg", name="osg")
                        g.wo_state = (n, m, osg)
                    eng = engines[g.ost_n % len(engines)]
                    g.ost_n += 1
                    if eng == "scalar":
                        nc.scalar.copy(osg[:, m, :], wps[:])
                    else:
                        nc.vector.tensor_copy(osg[:, m, :], wps[:])
                if m == 7:
                    if osg is not None:
                        if n == 7:
                            for q2 in range(4):
                                nc.sync.dma_start(
                                    io["outT"][:, 2 * q2 : 2 * q2 + 2, 512 * n : 512 * n + 512],
                                    osg[:, 2 * q2 : 2 * q2 + 2, :],
                                )
                        else:
                            nc.sync.dma_start(
                                io["outT"][:, :, 512 * n : 512 * n + 512], osg[:]
                            )
                    g.wo_state = None
                else:
                    g.wo_state = (n, m + 1, osg)

        # ---- projection, in quarter-chunk parts ------------------------
        def proj_parts(n):
            b, cn = divmod(n, 4)

            def part_qk(kind):
                def run():
                    xt = g.xt[n]
                    ps = psT.tile([128, 512], F32, tag="t", name="ps")
                    for e in range(8):
                        _lab(nc.tensor.matmul(
                            ps[:], g.w(kind, e), xt[:, e, :],
                            start=(e == 0), stop=(e == 7),
                        ), f"proj-{kind}-n{n}e{e}")
                    if kind == "wq":
                        o = 128 + 512 * cn
                        nc.scalar.copy(g.qTp[:, b, o : o + 512], ps[:])
                    else:
                        nc.vector.tensor_copy(g.kT[:, 512 * n : 512 * n + 512], ps[:])
                return run

            # v split in two parts sharing one psum tile; each 128-token
            # sub-block's 8-step accumulation is contiguous (u-outer)
            vps = [None]

            def part_v(half):
                def run():
                    xt = g.xt[n]
                    if half == 0:
                        vps[0] = psT.tile([128, 4, 128], F32, tag="t", name="psv")
                    psv = vps[0]
                    for u in range(2 * half, 2 * half + 2):
                        for e in range(8):
                            _lab(nc.tensor.matmul(
                                psv[:, u, :],
                                xt[:, e, 128 * u : 128 * u + 128],
                                g.w("wv", e),
                                start=(e == 0), stop=(e == 7),
                                skip_group_check=True,
                            ), f"proj-v-n{n}e{e}u{u}")
                    if half == 1:
                        nc.vector.tensor_copy(
                            g.vA[:, 4 * n : 4 * n + 4, :, 0:64], psv[:]
                        )
                return run

            return [part_qk("wq"), part_qk("wk"), part_v(0), part_v(1)]

        def proj_chunk(n):
            for p in proj_parts(n):
                p()

        # ---- attention stream for one batch (generator over j) ---------
        def attn_gen(b):
            def emit_score(j, h):
                # edge j-blocks skip their 128 padded columns entirely
                if j == 0:
                    qo, w, mk = 128, 256, mpack[:, 128:384]
                elif j == NB - 1:
                    qo, w, mk = 0, 256, mpack[:, 2 * WIN : 2 * WIN + 256]
                else:
                    qo, w, mk = 0, WIN, g.mask(1)
                sT = psA.tile([128, 512], F32, tag="sa", name="sT")
                _lab(nc.tensor.matmul(
                    sT[:, 0:w],
                    g.kT[64 * h : 64 * h + 64, S * b + 128 * j : S * b + 128 * j + 128],
                    g.qTp[64 * h : 64 * h + 64, b, 128 * j + qo : 128 * j + qo + w],
                    start=True, stop=True,
                ), f"score-b{b}j{j}h{h}")
                ex = expool.tile([128, WIN], BF16, tag="ex", name="ex")
                nc.scalar.activation(ex[:, 0:w], sT[:, 0:w], ACT_EXP)
                nc.vector.tensor_mul(ex[:, 0:w], ex[:, 0:w], mk)
                return ex

            def stage_a(c, u0, u1):
                rz0 = rzpool.tile([1, 512], BF16, tag="rz", name="rz0")
                rz1 = rzpool.tile([1, 512], BF16, tag="rz", name="rz1")
                with nc.allow_low_precision(reason="bf16 softmax scale"):
                    nc.vector.reciprocal(rz0[:], u0[64:65, :])
                    nc.vector.reciprocal(rz1[:], u1[64:65, :])
                dst = g.aoT[:, S * b + 512 * c : S * b + 512 * c + 512]
                nc.scalar.copy(dst[0:64, :], u0[0:64, :])
                return (c, u1, rz0, rz1)

            def stage_b(c, u1, rz0, rz1):
                dst = g.aoT[:, S * b + 512 * c : S * b + 512 * c + 512]
                nc.vector.tensor_copy(dst[64:128, :], u1[0:64, :])
                zr = psA.tile([128, 512], F32, tag="sa", name="zr")
                _lab(nc.tensor.matmul(zr[:], g.sel[:, 0:128], rz0[:],
                                      start=True, stop=False), f"zr-b{b}c{c}a")
                _lab(nc.tensor.matmul(zr[:], g.sel[:, 128:256], rz1[:],
                                      start=False, stop=True), f"zr-b{b}c{c}b")
                return (c, zr)

            def stage_c(c, zr):
                dst = g.aoT[:, S * b + 512 * c : S * b + 512 * c + 512]
                nc.vector.tensor_mul(dst, dst, zr[:])
                g.wo_ready.append(4 * b + c)

            def pv(j, h):
                qlo_w, qhi_w = 128 * (j - 1), 128 * (j + 2)
                ex_off = 0 if j == 0 else qlo_w
                tb = NB * b + j
                for c in sorted({max(qlo_w, 0) // 512, (min(qhi_w, S) - 1) // 512}):
                    plo = max(qlo_w, 512 * c, 0)
                    phi = min(qhi_w, 512 * c + 512, S)
                    if plo >= phi:
                        continue
                    if (h, c) not in umap:
                        umap[(h, c)] = psU.tile([65, 512], F32, tag="u", name="u")
                        fresh.add((h, c))
                    _lab(nc.tensor.matmul(
                        umap[(h, c)][:, plo - 512 * c : phi - 512 * c],
                        g.vA[:, tb, h, 0:65],
                        ex_cur[h][:, plo - ex_off : phi - ex_off],
                        start=((h, c) in fresh), stop=(j == min(4 * c + 4, NB - 1)),
                        skip_group_check=True,
                    ), f"pv-b{b}j{j}h{h}c{c}")
                    fresh.discard((h, c))

            umap, fresh, pend_b, pend_c = {}, set(), [], []
            ex_cur = [emit_score(0, h) for h in (0, 1)]
            for j in range(NB):
                while pend_c:
                    stage_c(*pend_c.pop(0))
                ex_nxt = [emit_score(j + 1, h) for h in (0, 1)] if j + 1 < NB else None
                pv(j, 0)
                yield ("mid", j)
                pv(j, 1)
                while pend_b:
                    pend_c.append(stage_b(*pend_b.pop(0)))
                for c in range(4):
                    if (0, c) in umap and j == min(4 * c + 4, NB - 1):
                        pend_b.append(stage_a(c, umap.pop((0, c)), umap.pop((1, c))))
                ex_cur = ex_nxt
                yield ("end", j)
            while pend_b:
                pend_c.append(stage_b(*pend_b.pop(0)))
            while pend_c:
                stage_c(*pend_c.pop(0))

        # ---- schedule ---------------------------------------------------
        # proj(0..3) whole; then batch-0 attention with proj(4..7) parts
        # as fillers (one quarter-chunk per j, between the PV groups);
        # then batch-1 attention with 2 wo drains per j; then tail.
        emit_x_dma(0, split=True)
        load_kv_weights()
        emit_x_dma(1)
        proj_chunk(0)
        emit_x_dma(2)
        proj_chunk(1)
        emit_x_dma(3)
        load_rest_consts()
        proj_chunk(2)
        emit_x_dma(4)
        proj_chunk(3)

        parts = []
        for n in range(4, 8):
            parts.extend(proj_parts(n))
        dma_at = {2: 5, 6: 6, 10: 7}
        for ev, j in attn_gen(0):
            if ev == "mid":
                if j in dma_at:
                    emit_x_dma(dma_at[j])
                parts[j]()
            else:
                if j >= 6:
                    drain_wo(2 if j < 10 else 3, engines=("scalar", "vector"))

        for ev, j in attn_gen(1):
            if ev == "mid":
                drain_wo(1, engines=("vector", "scalar"))
        for _ in range(40):
            drain_wo(2, engines=("scalar", "vector"), pools="both")
        if "dbgQ" in io:
            nc.sync.dma_start(io["dbgQ"][:], g.qTp[:])
            nc.sync.dma_start(io["dbgK"][:], g.kT[:])
            nc.sync.dma_start(io["dbgV"][:], g.vA[:])
            nc.sync.dma_start(io["dbgA"][:], g.aoT[:])


def build_program():
    WO_DMA_UNITS.clear()
    nc = bacc.Bacc("TRN2", target_bir_lowering=False, debug=False, num_devices=N_CORES)
    io = {}

    def inp(name, shape):
        io[name] = nc.dram_tensor(name, shape, BF16, kind="ExternalInput").ap()

    inp("xT", [128, 8, T])
    inp("wpack", [128, 4096])
    inp("mpack", [128, 3 * WIN])
    inp("sel", [1, 256])
    inp("qzero", [128, B, 2, 128])
    inp("vones", [128, 32, 2, 2])
    io["outT"] = nc.dram_tensor("outT", [128, 8, T], BF16, kind="ExternalOutput").ap()
    import os
    if os.environ.get("KDBG"):
        io["dbgQ"] = nc.dram_tensor("dbgQ", [128, B, PADW], BF16, kind="ExternalOutput").ap()
        io["dbgK"] = nc.dram_tensor("dbgK", [128, T], BF16, kind="ExternalOutput").ap()
        io["dbgV"] = nc.dram_tensor("dbgV", [128, 32, 2, 66], BF16, kind="ExternalOutput").ap()
        io["dbgA"] = nc.dram_tensor("dbgA", [128, T], BF16, kind="ExternalOutput").ap()
    io["outT32"] = nc.dram_tensor("outT32", [128, 8, T], F32, kind="ExternalOutput").ap()

    with tile.TileContext(nc) as tc:
        _emit(tc, io)
    nc.compile()
    return nc


def _to_bf16(a):
    u = np.ascontiguousarray(a, dtype=np.float32).view(np.uint32)
    r = ((u + 0x7FFF + ((u >> 16) & 1)) >> 16).astype(np.uint16)
    return r.view(ml_dtypes.bfloat16)


def _bf16_to_f32(a):
    u = np.asarray(a).view(np.uint16).astype(np.uint32) << 16
    return u.view(np.float32)


def _host_inputs(x, Wq, Wk, Wv, Wo):
    """Per-core input maps (host-side sharding / relayout)."""
    xf = np.ascontiguousarray(x.reshape(T, E).T)            # [1024, 4096]
    # [p, e, t]: E index = 128e + p
    xT = np.ascontiguousarray(_to_bf16(xf).reshape(8, 128, T).transpose(1, 0, 2))

    band = np.zeros((128, WIN), dtype=np.float32)
    for r in range(128):
        band[r, r : r + 257] = 1.0                           # |q - k| <= 128
    m_left = band.copy()
    m_left[:, :128] = 0.0
    m_right = band.copy()
    m_right[:, 256:] = 0.0
    mpack = _to_bf16(np.concatenate([m_left, band, m_right], axis=1))

    sel = np.zeros((1, 256), dtype=np.float32)
    sel[0, 0:64] = 1.0
    sel[0, 192:256] = 1.0
    sel = _to_bf16(sel)
    qzero = np.zeros((128, B, 2, 128), dtype=ml_dtypes.bfloat16)
    vones = np.ones((128, 32, 2, 2), dtype=ml_dtypes.bfloat16)

    scale = 1.0 / np.sqrt(D)
    in_maps = []
    for c in range(N_CORES):
        rows = slice(128 * c, 128 * c + 128)
        wq = np.ascontiguousarray((Wq[rows, :] * scale).T)   # [1024 e, 128 d']
        wk = np.ascontiguousarray(Wk[rows, :].T)
        wv = np.ascontiguousarray(Wv[rows, :].T)
        wqc = wq.reshape(8, 128, 128)
        wkc = wk.reshape(8, 128, 128)
        wvc = wv.reshape(8, 128, 128)
        woc = Wo[:, rows].T.reshape(128, 8, 128).transpose(1, 0, 2)  # [8,128 d',128 e]
        wpack = np.zeros((128, 4096), dtype=np.float32)
        for e in range(8):
            wpack[:, 0 + 128 * e : 128 * e + 128] = wqc[e]
            wpack[:, 1024 + 128 * e : 1152 + 128 * e] = wkc[e]
            wpack[:, 2048 + 128 * e : 2176 + 128 * e] = wvc[e]
            wpack[:, 3072 + 128 * e : 3200 + 128 * e] = woc[e]
        in_maps.append(
            {
                "xT": xT,
                "wpack": _to_bf16(wpack),
                "mpack": mpack,
                "sel": sel,
                "qzero": qzero,
                "vones": vones,
            }
        )
    return in_maps


_NC_CACHE = None


def kernel(x, Wq, Wk, Wv, Wo, bo):
    global _NC_CACHE
    x = np.asarray(x, dtype=np.float32)
    Wq = np.asarray(Wq, dtype=np.float32)
    Wk = np.asarray(Wk, dtype=np.float32)
    Wv = np.asarray(Wv, dtype=np.float32)
    Wo = np.asarray(Wo, dtype=np.float32)
    bo = np.asarray(bo, dtype=np.float32)

    if _NC_CACHE is None:
        _NC_CACHE = build_program()
    nc = _NC_CACHE

    in_maps = _host_inputs(x, Wq, Wk, Wv, Wo)
    res = run_bass_kernel_spmd(nc, in_maps, core_ids=list(range(N_CORES)))

    acc = np.zeros((E, T), dtype=np.float32)
    for c in range(N_CORES):
        # outT[p, m, t] -> E row = 128m + p
        o = _bf16_to_f32(res.results[c]["outT"]).reshape(128, 8, T)
        acc += o.transpose(1, 0, 2).reshape(E, T)
    out = acc.T + bo[None, :]
    return np.ascontiguousarray(out.reshape(B, S, E))


# revision 47
# speedup vs baseline: 1.0093x; 1.0023x over previous
"""Sliding-window multi-head attention on 8 Trainium2 NeuronCores.

Sharding: tensor-parallel over heads. 16 heads -> 2 heads per core.
Each core computes q/k/v projections for its 2 heads (d' = 128 dims),
banded (window=256) attention for those heads over all tokens, and a
partial output projection (its 128 columns of Wo). Host sums the 8
partials and adds the bias.

All matmul operands are bf16 (same PE throughput as f32r in the cost
model, half the DMA bytes, faster DVE); PSUM accumulation stays f32.
v is projected directly token-major (lhsT = x chunk, rhs = Wv chunk)
so no PE transposes are needed.

The attention phase is ACT-bound (64 exps ~32us vs ~22us of attention
PE work), so batch 1's projection chunks are split into quarter-chunk
"filler" parts and woven into batch 0's attention j-loop: every j gets
~1.3us of projection matmuls between the two heads' PV groups, keeping
PE ahead of ACT. Batch 0's Wo drains also run there; batch 1's drain
inside its own attention js plus a short 3-engine tail.

DMA instructions are fused (one per 512-token x chunk, one per
8-chunk output group; x/outT use a [128 partition, 8, T] layout so a
single DMA's dims line up) because each DMA costs ~0.6us serialized
HWDGE descriptor-generation plus ~0.6us SP-sequencer config.

Softmax normalization is three-staged across js: reciprocal+copy of
h0 when a u chunk completes, recip+copy h1 and the 1/Z broadcast
matmul one j later, the in-place scale at the top of the j after --
so the PE's in-order queue never waits on DVE latency.

PSUM (8 banks): psA x2 (score tiles + 1/Z broadcast), psU x4 (PV
accumulators), psT x2 (proj q/k/v dsts + wo chunks).
"""

import sys

sys.path.insert(0, "/opt/trn_rl_repo")

from contextlib import ExitStack

import numpy as np
import ml_dtypes

import concourse.bass as bass
import concourse.tile as tile
from concourse import bacc, mybir
from concourse.bass_utils import run_bass_kernel_spmd

F32 = mybir.dt.float32
BF16 = mybir.dt.bfloat16
ACT_EXP = mybir.ActivationFunctionType.Exp

N_CORES = 8
B, S, E = 2, 2048, 1024
H, D = 16, 64
T = B * S                # 4096 tokens total
NB = S // 128            # 16 key/query blocks per batch
PADW = S + 256           # 2304: padded q width per batch
WIN = 384                # q-window per key block (3 blocks)
WOFF = {"wq": 0, "wk": 1024, "wv": 2048, "wo": 3072}


class _Ctx:
    pass


LABELS = {}
WO_DMA_UNITS = set()


def _lab(inst, label):
    for obj in (inst, getattr(inst, "ins", None)):
        name = getattr(obj, "name", None)
        if isinstance(name, str):
            LABELS[name] = label
            break
    return inst


def _emit(tc, io):
    nc = tc.nc
    with ExitStack() as ctx:
        const = ctx.enter_context(tc.tile_pool(name="const", bufs=1))
        big = ctx.enter_context(tc.tile_pool(name="big", bufs=1))
        xpool = ctx.enter_context(tc.tile_pool(name="xload", bufs=4))
        expool = ctx.enter_context(tc.tile_pool(name="expool", bufs=10))
        rzpool = ctx.enter_context(tc.tile_pool(name="rzpool", bufs=6))
        ostage = ctx.enter_context(tc.tile_pool(name="ostage", bufs=4))
        psA = ctx.enter_context(tc.tile_pool(name="psA", bufs=2, space="PSUM"))
        psU = ctx.enter_context(tc.tile_pool(name="psU", bufs=4, space="PSUM"))
        psT = ctx.enter_context(tc.tile_pool(name="psT", bufs=2, space="PSUM"))

        g = _Ctx()

        # ---- constants (wq first so proj can start asap) ----------------
        wpack = const.tile([128, 4096], BF16, tag="wpack")
        nc.sync.dma_start(wpack[:, 0:256], io["wpack"][:, 0:256])
        nc.sync.dma_start(wpack[:, 256:1024], io["wpack"][:, 256:1024])
        mpack = const.tile([128, 3 * WIN], BF16, tag="mpack")
        sel = const.tile([1, 256], BF16, tag="sel")

        g.w = lambda kind, e: wpack[:, WOFF[kind] + 128 * e : WOFF[kind] + 128 * e + 128]
        g.mask = lambda i: mpack[:, WIN * i : WIN * i + WIN]
        g.sel = sel

        # ---- persistent activation buffers -----------------------------
        g.qTp = big.tile([128, B, PADW], BF16, tag="qTp")
        g.kT = big.tile([128, T], BF16, tag="kT")
        # vA: [128 tok-in-block, 32 tok-blocks, 2 heads, 64 dims + 2 ones]
        g.vA = big.tile([128, 32, 2, 66], BF16, tag="vA")
        g.aoT = big.tile([128, T], BF16, tag="aoT")

        def load_kv_weights():
            nc.sync.dma_start(wpack[:, 1024:2048], io["wpack"][:, 1024:2048])
            nc.sync.dma_start(wpack[:, 2048:3072], io["wpack"][:, 2048:3072])

        def load_rest_consts():
            nc.sync.dma_start(wpack[:, 3072:4096], io["wpack"][:, 3072:4096])
            nc.sync.dma_start(mpack[:], io["mpack"][:])
            nc.sync.dma_start(sel[:], io["sel"][:])
            nc.sync.dma_start(g.qTp[:, :, 0:128], io["qzero"][:, :, 0, :])
            nc.sync.dma_start(g.qTp[:, :, S + 128 : PADW], io["qzero"][:, :, 1, :])
            nc.sync.dma_start(g.vA[:, :, :, 64:66], io["vones"][:])

        # ---- x chunk loads ---------------------------------------------
        g.xt = {}

        def emit_x_dma(n, split=False):
            xt = xpool.tile([128, 8, 512], BF16, tag="xt", name="xt")
            if split:
                queues = (nc.scalar, nc.scalar, nc.sync, nc.sync)
                for i, e in enumerate(range(0, 8, 2)):
                    queues[i].dma_start(
                        xt[:, e : e + 2, :], io["xT"][:, e : e + 2, 512 * n : 512 * n + 512]
                    )
            else:
                nc.sync.dma_start(xt[:], io["xT"][:, :, 512 * n : 512 * n + 512])
            g.xt[n] = xt

        # ---- wo drain: one osg staging tile + one fused DMA per n ------
        g.wo_ready = []          # list of n (each is 8 m-units)
        g.wo_state = None        # (n, next_m, osg_tile)
        g.ost_n = 0

        def drain_wo(k, engines=("vector", "scalar"), mode="copy", pools=None):
            for _ in range(k):
                if g.wo_state is None:
                    if not g.wo_ready:
                        return
                    n = g.wo_ready.pop(0)
                    osg = None
                    g.wo_state = (n, 0, osg)
                n, m, osg = g.wo_state
                if pools is None:
                    wps = psT.tile([128, 512], F32, tag="t", name="wps")
                elif (g.ost_n // 1) % 2:
                    wps = psA.tile([128, 512], F32, tag="sa", name="wps")
                else:
                    wps = psT.tile([128, 512], F32, tag="t", name="wps")
                _lab(nc.tensor.matmul(
                    wps[:], g.w("wo", m), g.aoT[:, 512 * n : 512 * n + 512],
                    start=True, stop=True,
                ), f"wo-n{n}m{m}")
                if mode == "dma":
                    # drain straight from PSUM to dram (f32) on an idle
                    # sequencer queue; no ACT/DVE copy work
                    WO_DMA_UNITS.add((n, m))
                    q = nc.scalar if (g.ost_n % 2) else nc.vector
                    g.ost_n += 1
                    q.dma_start(
                        io["outT32"][:, m, 512 * n : 512 * n + 512], wps[:]
                    )
                else:
                    if osg is None:
                        osg = ostage.tile([128, 8, 512], BF16, tag="osg", name="osg")
                        g.wo_state = (n, m, osg)
                    eng = engines[g.ost_n % len(engines)]
                    g.ost_n += 1
                    if eng == "scalar":
                        nc.scalar.copy(osg[:, m, :], wps[:])
                    else:
                        nc.vector.tensor_copy(osg[:, m, :], wps[:])
                if m == 7:
                    if osg is not None:
                        if n == 7:
                            for q2 in range(4):
                                nc.sync.dma_start(
                                    io["outT"][:, 2 * q2 : 2 * q2 + 2, 512 * n : 512 * n + 512],
                                    osg[:, 2 * q2 : 2 * q2 + 2, :],
                                )
                        else:
                            nc.sync.dma_start(
                                io["outT"][:, :, 512 * n : 512 * n + 512], osg[:]
                            )
                    g.wo_state = None
                else:
                    g.wo_state = (n, m + 1, osg)

        # ---- projection, in quarter-chunk parts ------------------------
        def proj_parts(n):
            b, cn = divmod(n, 4)

            def part_qk(kind):
                def run():
                    xt = g.xt[n]
                    ps = psT.tile([128, 512], F32, tag="t", name="ps")
                    for e in range(8):
                        _lab(nc.tensor.matmul(
                            ps[:], g.w(kind, e), xt[:, e, :],
                            start=(e == 0), stop=(e == 7),
                        ), f"proj-{kind}-n{n}e{e}")
                    if kind == "wq":
                        o = 128 + 512 * cn
                        nc.scalar.copy(g.qTp[:, b, o : o + 512], ps[:])
                    else:
                        nc.vector.tensor_copy(g.kT[:, 512 * n : 512 * n + 512], ps[:])
                return run

            # v split in two parts sharing one psum tile; each 128-token
            # sub-block's 8-step accumulation is contiguous (u-outer)
            vps = [None]

            def part_v(half):
                def run():
                    xt = g.xt[n]
                    if half == 0:
                        vps[0] = psT.tile([128, 4, 128], F32, tag="t", name="psv")
                    psv = vps[0]
                    for u in range(2 * half, 2 * half + 2):
                        for e in range(8):
                            _lab(nc.tensor.matmul(
                                psv[:, u, :],
                                xt[:, e, 128 * u : 128 * u + 128],
                                g.w("wv", e),
                                start=(e == 0), stop=(e == 7),
                                skip_group_check=True,
                            ), f"proj-v-n{n}e{e}u{u}")
                    if half == 1:
                        nc.vector.tensor_copy(
                            g.vA[:, 4 * n : 4 * n + 4, :, 0:64], psv[:]
                        )
                return run

            return [part_qk("wq"), part_qk("wk"), part_v(0), part_v(1)]

        def proj_chunk(n):
            for p in proj_parts(n):
                p()

        # ---- attention stream for one batch (generator over j) ---------
        def attn_gen(b):
            def emit_score(j, h):
                # edge j-blocks skip their 128 padded columns entirely
                if j == 0:
                    qo, w, mk = 128, 256, mpack[:, 128:384]
                elif j == NB - 1:
                    qo, w, mk = 0, 256, mpack[:, 2 * WIN : 2 * WIN + 256]
                else:
                    qo, w, mk = 0, WIN, g.mask(1)
                sT = psA.tile([128, 512], F32, tag="sa", name="sT")
                _lab(nc.tensor.matmul(
                    sT[:, 0:w],
                    g.kT[64 * h : 64 * h + 64, S * b + 128 * j : S * b + 128 * j + 128],
                    g.qTp[64 * h : 64 * h + 64, b, 128 * j + qo : 128 * j + qo + w],
                    start=True, stop=True,
                ), f"score-b{b}j{j}h{h}")
                ex = expool.tile([128, WIN], BF16, tag="ex", name="ex")
                nc.scalar.activation(ex[:, 0:w], sT[:, 0:w], ACT_EXP)
                nc.vector.tensor_mul(ex[:, 0:w], ex[:, 0:w], mk)
                return ex

            def stage_a(c, u0, u1):
                rz0 = rzpool.tile([1, 512], BF16, tag="rz", name="rz0")
                rz1 = rzpool.tile([1, 512], BF16, tag="rz", name="rz1")
                with nc.allow_low_precision(reason="bf16 softmax scale"):
                    nc.vector.reciprocal(rz0[:], u0[64:65, :])
                    nc.vector.reciprocal(rz1[:], u1[64:65, :])
                dst = g.aoT[:, S * b + 512 * c : S * b + 512 * c + 512]
                nc.scalar.copy(dst[0:64, :], u0[0:64, :])
                return (c, u1, rz0, rz1)

            def stage_b(c, u1, rz0, rz1):
                dst = g.aoT[:, S * b + 512 * c : S * b + 512 * c + 512]
                nc.vector.tensor_copy(dst[64:128, :], u1[0:64, :])
                zr = psA.tile([128, 512], F32, tag="sa", name="zr")
                _lab(nc.tensor.matmul(zr[:], g.sel[:, 0:128], rz0[:],
                                      start=True, stop=False), f"zr-b{b}c{c}a")
                _lab(nc.tensor.matmul(zr[:], g.sel[:, 128:256], rz1[:],
                                      start=False, stop=True), f"zr-b{b}c{c}b")
                return (c, zr)

            def stage_c(c, zr):
                dst = g.aoT[:, S * b + 512 * c : S * b + 512 * c + 512]
                nc.vector.tensor_mul(dst, dst, zr[:])
                g.wo_ready.append(4 * b + c)

            def pv(j, h):
                qlo_w, qhi_w = 128 * (j - 1), 128 * (j + 2)
                ex_off = 0 if j == 0 else qlo_w
                tb = NB * b + j
                for c in sorted({max(qlo_w, 0) // 512, (min(qhi_w, S) - 1) // 512}):
                    plo = max(qlo_w, 512 * c, 0)
                    phi = min(qhi_w, 512 * c + 512, S)
                    if plo >= phi:
                        continue
                    if (h, c) not in umap:
                        umap[(h, c)] = psU.tile([65, 512], F32, tag="u", name="u")
                        fresh.add((h, c))
                    _lab(nc.tensor.matmul(
                        umap[(h, c)][:, plo - 512 * c : phi - 512 * c],
                        g.vA[:, tb, h, 0:65],
                        ex_cur[h][:, plo - ex_off : phi - ex_off],
                        start=((h, c) in fresh), stop=(j == min(4 * c + 4, NB - 1)),
                        skip_group_check=True,
                    ), f"pv-b{b}j{j}h{h}c{c}")
                    fresh.discard((h, c))

            umap, fresh, pend_b, pend_c = {}, set(), [], []
            ex_cur = [emit_score(0, h) for h in (0, 1)]
            for j in range(NB):
                while pend_c:
                    stage_c(*pend_c.pop(0))
                ex_nxt = [emit_score(j + 1, h) for h in (0, 1)] if j + 1 < NB else None
                pv(j, 0)
                yield ("mid", j)
                pv(j, 1)
                while pend_b:
                    pend_c.append(stage_b(*pend_b.pop(0)))
                for c in range(4):
                    if (0, c) in umap and j == min(4 * c + 4, NB - 1):
                        pend_b.append(stage_a(c, umap.pop((0, c)), umap.pop((1, c))))
                ex_cur = ex_nxt
                yield ("end", j)
            while pend_b:
                pend_c.append(stage_b(*pend_b.pop(0)))
            while pend_c:
                stage_c(*pend_c.pop(0))

        # ---- schedule ---------------------------------------------------
        # proj(0..3) whole; then batch-0 attention with proj(4..7) parts
        # as fillers (one quarter-chunk per j, between the PV groups);
        # then batch-1 attention with 2 wo drains per j; then tail.
        emit_x_dma(0, split=True)
        load_kv_weights()
        emit_x_dma(1)
        proj_chunk(0)
        emit_x_dma(2)
        proj_chunk(1)
        emit_x_dma(3)
        load_rest_consts()
        proj_chunk(2)
        emit_x_dma(4)
        proj_chunk(3)

        parts = []
        for n in range(4, 8):
            parts.extend(proj_parts(n))
        dma_at = {2: 5, 6: 6, 10: 7}
        for ev, j in attn_gen(0):
            if ev == "mid":
                if j in dma_at:
                    emit_x_dma(dma_at[j])
                if j < 14:
                    parts[j]()
            else:
                if j >= 6:
                    drain_wo(2 if j < 10 else 3, engines=("scalar", "vector"))

        for ev, j in attn_gen(1):
            if ev == "mid":
                if j < 2:
                    parts[14 + j]()
                drain_wo(1, engines=("vector", "scalar"))
        for _ in range(40):
            drain_wo(2, engines=("scalar", "vector"), pools="both")
        if "dbgQ" in io:
            nc.sync.dma_start(io["dbgQ"][:], g.qTp[:])
            nc.sync.dma_start(io["dbgK"][:], g.kT[:])
            nc.sync.dma_start(io["dbgV"][:], g.vA[:])
            nc.sync.dma_start(io["dbgA"][:], g.aoT[:])


def build_program():
    WO_DMA_UNITS.clear()
    nc = bacc.Bacc("TRN2", target_bir_lowering=False, debug=False, num_devices=N_CORES)
    io = {}

    def inp(name, shape):
        io[name] = nc.dram_tensor(name, shape, BF16, kind="ExternalInput").ap()

    inp("xT", [128, 8, T])
    inp("wpack", [128, 4096])
    inp("mpack", [128, 3 * WIN])
    inp("sel", [1, 256])
    inp("qzero", [128, B, 2, 128])
    inp("vones", [128, 32, 2, 2])
    io["outT"] = nc.dram_tensor("outT", [128, 8, T], BF16, kind="ExternalOutput").ap()
    import os
    if os.environ.get("KDBG"):
        io["dbgQ"] = nc.dram_tensor("dbgQ", [128, B, PADW], BF16, kind="ExternalOutput").ap()
        io["dbgK"] = nc.dram_tensor("dbgK", [128, T], BF16, kind="ExternalOutput").ap()
        io["dbgV"] = nc.dram_tensor("dbgV", [128, 32, 2, 66], BF16, kind="ExternalOutput").ap()
        io["dbgA"] = nc.dram_tensor("dbgA", [128, T], BF16, kind="ExternalOutput").ap()
    io["outT32"] = nc.dram_tensor("outT32", [128, 8, T], F32, kind="ExternalOutput").ap()

    with tile.TileContext(nc) as tc:
        _emit(tc, io)
    nc.compile()
    return nc


def _to_bf16(a):
    u = np.ascontiguousarray(a, dtype=np.float32).view(np.uint32)
    r = ((u + 0x7FFF + ((u >> 16) & 1)) >> 16).astype(np.uint16)
    return r.view(ml_dtypes.bfloat16)


def _bf16_to_f32(a):
    u = np.asarray(a).view(np.uint16).astype(np.uint32) << 16
    return u.view(np.float32)


def _host_inputs(x, Wq, Wk, Wv, Wo):
    """Per-core input maps (host-side sharding / relayout)."""
    xf = np.ascontiguousarray(x.reshape(T, E).T)            # [1024, 4096]
    # [p, e, t]: E index = 128e + p
    xT = np.ascontiguousarray(_to_bf16(xf).reshape(8, 128, T).transpose(1, 0, 2))

    band = np.zeros((128, WIN), dtype=np.float32)
    for r in range(128):
        band[r, r : r + 257] = 1.0                           # |q - k| <= 128
    m_left = band.copy()
    m_left[:, :128] = 0.0
    m_right = band.copy()
    m_right[:, 256:] = 0.0
    mpack = _to_bf16(np.concatenate([m_left, band, m_right], axis=1))

    sel = np.zeros((1, 256), dtype=np.float32)
    sel[0, 0:64] = 1.0
    sel[0, 192:256] = 1.0
    sel = _to_bf16(sel)
    qzero = np.zeros((128, B, 2, 128), dtype=ml_dtypes.bfloat16)
    vones = np.ones((128, 32, 2, 2), dtype=ml_dtypes.bfloat16)

    scale = 1.0 / np.sqrt(D)
    in_maps = []
    for c in range(N_CORES):
        rows = slice(128 * c, 128 * c + 128)
        wq = np.ascontiguousarray((Wq[rows, :] * scale).T)   # [1024 e, 128 d']
        wk = np.ascontiguousarray(Wk[rows, :].T)
        wv = np.ascontiguousarray(Wv[rows, :].T)
        wqc = wq.reshape(8, 128, 128)
        wkc = wk.reshape(8, 128, 128)
        wvc = wv.reshape(8, 128, 128)
        woc = Wo[:, rows].T.reshape(128, 8, 128).transpose(1, 0, 2)  # [8,128 d',128 e]
        wpack = np.zeros((128, 4096), dtype=np.float32)
        for e in range(8):
            wpack[:, 0 + 128 * e : 128 * e + 128] = wqc[e]
            wpack[:, 1024 + 128 * e : 1152 + 128 * e] = wkc[e]
            wpack[:, 2048 + 128 * e : 2176 + 128 * e] = wvc[e]
            wpack[:, 3072 + 128 * e : 3200 + 128 * e] = woc[e]
        in_maps.append(
            {
                "xT": xT,
                "wpack": _to_bf16(wpack),
                "mpack": mpack,
                "sel": sel,
                "qzero": qzero,
                "vones": vones,
            }
        )
    return in_maps


_NC_CACHE = None


def kernel(x, Wq, Wk, Wv, Wo, bo):
    global _NC_CACHE
    x = np.asarray(x, dtype=np.float32)
    Wq = np.asarray(Wq, dtype=np.float32)
    Wk = np.asarray(Wk, dtype=np.float32)
    Wv = np.asarray(Wv, dtype=np.float32)
    Wo = np.asarray(Wo, dtype=np.float32)
    bo = np.asarray(bo, dtype=np.float32)

    if _NC_CACHE is None:
        _NC_CACHE = build_program()
    nc = _NC_CACHE

    in_maps = _host_inputs(x, Wq, Wk, Wv, Wo)
    res = run_bass_kernel_spmd(nc, in_maps, core_ids=list(range(N_CORES)))

    acc = np.zeros((E, T), dtype=np.float32)
    for c in range(N_CORES):
        # outT[p, m, t] -> E row = 128m + p
        o = _bf16_to_f32(res.results[c]["outT"]).reshape(128, 8, T)
        acc += o.transpose(1, 0, 2).reshape(E, T)
    out = acc.T + bo[None, :]
    return np.ascontiguousarray(out.reshape(B, S, E))
